# revision 1
# baseline (speedup 1.0000x reference)
"""Multi-head attention + residual + LayerNorm, Trainium2 Bass kernel.

Problem (hardcoded): B=8, S=2048, D=512, H=8, DK=64, fp32 I/O.
  q = Q@Wq.T+bq; k = K@Wk.T+bk; v = V@Wv.T+bv        (per batch, split 8 heads)
  attn = softmax(q k^T / sqrt(DK)); ctx = attn @ v
  out = LayerNorm(ctx@Wo.T + bo + Q) * gamma + beta

Sharding: pure data-parallel over batch: core b handles batch element b
(B == n_cores == 8), no collectives.

Per-core dataflow (t-major attention, bf16 matmuls, fp32 LN):
  - Q/K/V pre-transposed on host to k-major bf16 [D, S]; loaded as
    chunks QT/KT/VT [128k, 2048s].
  - Projections on PE: qT,kT in [d_out, s] layout; v in natural [t, d_v].
  - Per head-pair (2 heads share a 128-partition chunk):
      scoresT[t,s] = kT^T@qT via row-packed (K=64) pair of matmuls,
      exp via ACT (true exp, head A) and DVE (Schraudolph bf16 bit-trick,
      head B), unnormalized probs kept bf16,
      ctxT[dv,s] += v^T-slice @ attnT via col-packed pair (heads at
      partitions 0:64 / 64:128), denominators via M=1 ones-matmuls.
  - Normalization folded in at the end: 1/den broadcast, ctxT *= recip.
  - Output projection back to natural [s, d] + bias (K=1 ones-matmul),
    + residual via fp32r identity-matmul into the same PSUM accumulator,
    LayerNorm (bn_stats/bn_aggr on PSUM, ACT applies (x-mu)*rstd),
    gamma/beta, DMA out.

Toolchain workarounds: this walrus build caps sem-waits per instruction
at 1 (excess waits hoisted onto same-engine NOPs) and rejects custom-DVE
ops (reciprocal done as exp(-ln(x)) on ACT).
"""

import numpy as np
import ml_dtypes

import bass_rust
import concourse.bass as bass
import concourse.mybir as mybir
import concourse.tile as tile
from concourse.bass_utils import run_bass_kernel_spmd
from concourse.vector_clock import ScopedClock

F32 = mybir.dt.float32
F32R = mybir.dt.float32r
BF16 = mybir.dt.bfloat16
I16 = mybir.dt.int16
AF = mybir.ActivationFunctionType
OP = mybir.AluOpType

N_CORES = 8
S, D, H, DK = 2048, 512, 8, 64
P = 128
KC = D // P        # 4 contraction chunks
TC = S // P        # 16 t-chunks
ST = S // P        # 16 s-tiles (output)
SBW = 512          # attention s-block width
NSB = S // SBW     # 4
EPS = 1e-5
SCALE = 1.0 / np.sqrt(DK)

# Schraudolph exp in bf16-bit space: bits = round(x*L*SCALE + (16256 - C))
SCH_L = 128.0 / np.log(2.0)
SCH_C = 5.60
SCH_S = float(SCALE * SCH_L)
SCH_B = float(16256.0 - SCH_C)

_MAX_CTRL_WAITS = 1


def _patch_tile_tail():
    """walrus in this toolchain rejects >1 sem wait on CTRL instructions
    (Drain/NoOp). Move the Tile tail-drain's waits onto a chain of NOPs,
    one wait each."""
    if getattr(tile.TileContext, "_tail_patched", False):
        return

    def _patched(self, tick_clock, wait_clock):
        nc = self.nc
        scratch = nc.sync.nop(nofuse=True, hint="tail_wait")
        wait_clock.add_sem_waits(
            scratch.ins, ScopedClock({None: tick_clock.global_clock})
        )
        si = scratch.ins.sync_info
        waits = list(si.on_wait) if si is not None else []
        if len(waits) > _MAX_CTRL_WAITS:
            scratch.ins.sync_info = bass_rust.SyncInfo(
                on_wait=waits[:_MAX_CTRL_WAITS], on_update=list(si.on_update)
            )
            for i in range(_MAX_CTRL_WAITS, len(waits), _MAX_CTRL_WAITS):
                extra = nc.sync.nop(nofuse=True, hint=f"tail_wait_{i}")
                extra.ins.sync_info = bass_rust.SyncInfo(
                    on_wait=waits[i : i + _MAX_CTRL_WAITS], on_update=[]
                )
        nc.sync.drain()
        nc.all_engine_barrier()
        popped = nc._tile_sem_poison_stack.pop()
        assert popped is self._sem_poison
        nc.clear_and_free_semaphores(list(self.sems.allocated().values()))
        nc.all_engine_barrier()

    tile.TileContext._drain_and_barrier = _patched
    tile.TileContext._tail_patched = True


def _split_excess_waits(nc, max_waits=_MAX_CTRL_WAITS):
    """walrus (this build) caps sem waits per instruction very low. Hoist
    excess waits onto same-engine NOPs inserted just before the instruction
    (same queue, in order — semantically identical)."""
    def make_nop(engine, waits):
        bi = nc.engines[engine].nop(nofuse=True, hint="waitsplit")
        nop_inst = bi.ins
        cur = nc.cur_bb.bb
        lst = list(cur.instructions)
        assert lst and lst[-1].name == nop_inst.name
        lst.pop()
        cur.instructions = lst
        nop_inst.sync_info = bass_rust.SyncInfo(on_wait=waits, on_update=[])
        return nop_inst

    ctr = 0
    for f in nc.m.functions:
        for bb in f.blocks:
            old = list(bb.instructions)
            new = []
            changed = False
            for inst in old:
                si = inst.sync_info
                waits = list(si.on_wait) if si is not None else []
                if len(waits) > max_waits:
                    changed = True
                    excess, keep = waits[:-max_waits], waits[-max_waits:]
                    for i in range(0, len(excess), max_waits):
                        ctr += 1
                        new.append(make_nop(inst.engine, excess[i : i + max_waits]))
                    inst.sync_info = bass_rust.SyncInfo(
                        on_wait=keep, on_update=list(si.on_update)
                    )
                new.append(inst)
            if changed:
                bb.instructions = new
    return ctr


_LDW_OPT = False


def _patch_ldw_opt():
    """Enable walrus's LDWEIGHTS pull-ahead (background weight buffer) —
    concourse pins it off, but it is a large win for our LDW-per-matmul
    phase C. Correctness is re-verified against the reference each run."""
    import concourse.bass_utils as bu

    if getattr(bu, "_ldw_patched", False):
        return
    orig = bu.run_command

    def patched(cmd, **kw):
        if _LDW_OPT and isinstance(cmd, list):
            cmd = [
                c.replace("--enable-ldw-opt=false", "--enable-ldw-opt=true")
                if isinstance(c, str)
                else c
                for c in cmd
            ]
        return orig(cmd, **kw)

    bu.run_command = patched
    bu._ldw_patched = True


def build_program(phases="ABCD"):
    _patch_tile_tail()
    _patch_ldw_opt()
    nc = bass.Bass("TRN2", target_bir_lowering=False, debug=False, num_devices=1)

    qf = nc.dram_tensor("qf", (S, D), F32R, kind="ExternalInput").ap()
    qb = nc.dram_tensor("qb", (D, S), BF16, kind="ExternalInput").ap()
    kb = nc.dram_tensor("kb", (D, S), BF16, kind="ExternalInput").ap()
    vb = nc.dram_tensor("vb", (D, S), BF16, kind="ExternalInput").ap()
    wq = nc.dram_tensor("wq", (D, D), BF16, kind="ExternalInput").ap()
    wk = nc.dram_tensor("wk", (D, D), BF16, kind="ExternalInput").ap()
    wv = nc.dram_tensor("wv", (D, D), BF16, kind="ExternalInput").ap()
    wo = nc.dram_tensor("wo", (D, D), BF16, kind="ExternalInput").ap()
    bq = nc.dram_tensor("bq", (D,), F32, kind="ExternalInput").ap()
    bk = nc.dram_tensor("bk", (D,), F32, kind="ExternalInput").ap()
    bv = nc.dram_tensor("bv", (1, D), BF16, kind="ExternalInput").ap()
    bo = nc.dram_tensor("bo", (1, D), BF16, kind="ExternalInput").ap()
    gamma = nc.dram_tensor("gamma", (D,), F32, kind="ExternalInput").ap()
    beta = nc.dram_tensor("beta", (D,), F32, kind="ExternalInput").ap()
    ident = nc.dram_tensor("ident", (P, P), F32R, kind="ExternalInput").ap()
    out = nc.dram_tensor("out", (S, D), F32, kind="ExternalOutput").ap()
    # DRAM scratch for the per-(pair, s-block) softmax-recip rows: written
    # bf16-cast, read back partition-broadcast (SBUF APs can't 0-step).
    dscr = nc.dram_tensor("dscr", (KC, 2, S), BF16, kind="Internal").ap()

    with tile.TileContext(nc) as tc:
        with tc.tile_pool(name="persist", bufs=1) as pp:
            # ---- constants / weights ----
            W_sb = {}
            for wname, wap in (("wq", wq), ("wk", wk), ("wv", wv), ("wo", wo)):
                for c in range(KC):
                    t = pp.tile([P, D], BF16, name=f"{wname}{c}")
                    nc.gpsimd.dma_start(out=t, in_=wap[c * P : (c + 1) * P, :])
                    W_sb[wname, c] = t
            bq_sb, bk_sb = [], []
            for c in range(KC):
                t = pp.tile([P, 1], F32, name=f"bq{c}")
                nc.gpsimd.dma_start(out=t, in_=bq[c * P : (c + 1) * P].unsqueeze(1))
                bq_sb.append(t)
                t = pp.tile([P, 1], F32, name=f"bk{c}")
                nc.gpsimd.dma_start(out=t, in_=bk[c * P : (c + 1) * P].unsqueeze(1))
                bk_sb.append(t)
            bv_sb = pp.tile([1, D], BF16, name="bv")
            nc.gpsimd.dma_start(out=bv_sb, in_=bv)
            bo_sb = pp.tile([1, D], BF16, name="bo")
            nc.gpsimd.dma_start(out=bo_sb, in_=bo)
            gamma_sb = pp.tile([P, D], F32, name="gamma")
            nc.gpsimd.dma_start(out=gamma_sb, in_=gamma.unsqueeze(0).broadcast_to([P, D]))
            beta_sb = pp.tile([P, D], F32, name="beta")
            nc.gpsimd.dma_start(out=beta_sb, in_=beta.unsqueeze(0).broadcast_to([P, D]))
            eps_sb = pp.tile([P, 1], F32, name="eps")
            nc.vector.memset(eps_sb, EPS)
            ones_t = pp.tile([P, 1], BF16, name="ones_t")
            nc.vector.memset(ones_t, 1.0)
            ones_r = pp.tile([1, P], BF16, name="ones_r")
            nc.vector.memset(ones_r, 1.0)
            ident_sb = pp.tile([P, P], F32R, name="ident")
            nc.sync.dma_start(out=ident_sb, in_=ident)

            # ---- phase A: k-major input loads (host pre-transposed) ----
            QT, KT, VT = [], [], []
            for lst, srcap, nm, eng in (
                (QT, qb, "QT", nc.sync),
                (KT, kb, "KT", nc.scalar),
                (VT, vb, "VT", nc.gpsimd),
            ):
                for c in range(KC):
                    t = pp.tile([P, S], BF16, name=f"{nm}{c}")
                    eng.dma_start(out=t, in_=srcap[c * P : (c + 1) * P, :])
                    lst.append(t)

            # ---- phase B: projections ----
            qTp = [pp.tile([P, S], BF16, name=f"qTp{c}") for c in range(KC)]
            kTp = [pp.tile([P, S], BF16, name=f"kTp{c}") for c in range(KC)]
            v_sb = [pp.tile([P, D], BF16, name=f"v{t}") for t in range(TC)]

            with tc.tile_pool(name="psum_b", bufs=4, space="PSUM") as ppool:
              if "B" in phases:
                  for c in range(KC):
                      for sbh in range(S // 512):
                          ssl = bass.ts(sbh, 512)
                          pq = ppool.tile([P, 512], F32, name="proj")
                          for kc in range(KC):
                              nc.tensor.matmul(
                                  pq,
                                  lhsT=W_sb["wq", kc][:, c * P : (c + 1) * P],
                                  rhs=QT[kc][:, ssl],
                                  start=(kc == 0),
                                  stop=(kc == KC - 1),
                              )
                          nc.scalar.activation(
                              out=qTp[c][:, ssl], in_=pq, func=AF.Identity, bias=bq_sb[c]
                          )
                          pk = ppool.tile([P, 512], F32, name="proj")
                          for kc in range(KC):
                              nc.tensor.matmul(
                                  pk,
                                  lhsT=W_sb["wk", kc][:, c * P : (c + 1) * P],
                                  rhs=KT[kc][:, ssl],
                                  start=(kc == 0),
                                  stop=(kc == KC - 1),
                              )
                          nc.vector.tensor_scalar(
                              out=kTp[c][:, ssl],
                              in0=pk,
                              scalar1=bk_sb[c],
                              scalar2=None,
                              op0=OP.add,
                          )
                  for t in range(TC):
                      pv = ppool.tile([P, 512], F32, name="proj")
                      for kc in range(KC):
                          nc.tensor.matmul(
                              pv,
                              lhsT=VT[kc][:, t * P : (t + 1) * P],
                              rhs=W_sb["wv", kc],
                              start=(kc == 0),
                              stop=False,
                          )
                      nc.tensor.matmul(pv, lhsT=ones_r, rhs=bv_sb, start=False, stop=True)
                      nc.vector.tensor_copy(out=v_sb[t], in_=pv)

            # ---- phase C: attention (per head pair p: heads 2p, 2p+1) ----
            ctxT = [pp.tile([P, S], BF16, name=f"ctxT{c}") for c in range(KC)]
            rb = [pp.tile([P, S], BF16, name=f"rb{c}") for c in range(KC)]

            with (
                tc.tile_pool(name="psum_sc", bufs=2, space="PSUM") as psc,
                tc.tile_pool(name="psum_cd", bufs=2, space="PSUM") as pcd,
                tc.tile_pool(name="attn", bufs=4) as apool,
                tc.tile_pool(name="denr", bufs=2) as dpool,
            ):
              if "C" in phases:
                  def _boundary(bp, bsb, bctx, bden):
                      # drain a finished (pair, s-block): ctx copyback,
                      # 1/den = exp(-ln(den)) on ACT, DRAM-roundtrip bcast.
                      # Called 2 t-chunks into the NEXT block so this work
                      # queues BEHIND that block's first exps and no longer
                      # stalls them (the PE gap at every block boundary).
                      bsl = bass.ts(bsb, SBW)
                      nc.vector.tensor_copy(out=ctxT[bp][:, bsl], in_=bctx)
                      lnt = dpool.tile([33, SBW], F32, name="lnt")
                      nc.scalar.activation(out=lnt, in_=bden[0:33, :], func=AF.Ln)
                      stage = dpool.tile([33, SBW], F32, name="denr")
                      nc.scalar.activation(
                          out=stage, in_=lnt, func=AF.Exp, scale=-1.0
                      )
                      dsl = bass.ds(bsb * SBW, SBW)
                      nc.gpsimd.dma_start(out=dscr[bp, 0, dsl], in_=stage[0:1, :])
                      nc.gpsimd.dma_start(out=dscr[bp, 1, dsl], in_=stage[32:33, :])
                      nc.gpsimd.dma_start(
                          out=rb[bp][0:64, bsl],
                          in_=dscr[bp, 0, dsl].unsqueeze(0).broadcast_to([64, SBW]),
                      )
                      nc.gpsimd.dma_start(
                          out=rb[bp][64:P, bsl],
                          in_=dscr[bp, 1, dsl].unsqueeze(0).broadcast_to([64, SBW]),
                      )
                      if bsb == NSB - 1:
                          # fold softmax 1/den into ctxT while later work
                          # still runs (keeps PE warm into the projection)
                          nc.vector.tensor_mul(
                              out=ctxT[bp], in0=ctxT[bp], in1=rb[bp]
                          )

                  _pending = [None]
                  for p in range(KC):
                      dvA = bass.ds(p * P, DK)
                      dvB = bass.ds(p * P + DK, DK)
                      for sb in range(NSB):
                          ctx_ps = pcd.tile([P, SBW], F32, name="ctx")
                          den_ps = pcd.tile([P, SBW], F32, name="den")
                          for tci in range(TC):
                              if tci == 2 and _pending[0] is not None:
                                  _boundary(*_pending[0])
                                  _pending[0] = None
                              tsl = bass.ts(tci, P)
                              scA = psc.tile([P, SBW], F32, name="scA")
                              scB = psc.tile([P, SBW], F32, name="scB")
                              qsl = bass.ds(sb * SBW, SBW)
                              nc.tensor.matmul(
                                  scA,
                                  lhsT=kTp[p][0:DK, tsl],
                                  rhs=qTp[p][0:DK, qsl],
                                  start=True,
                                  stop=True,
                                  tile_position=(0, 0),
                              )
                              nc.tensor.matmul(
                                  scB,
                                  lhsT=kTp[p][DK:P, tsl],
                                  rhs=qTp[p][DK:P, qsl],
                                  start=True,
                                  stop=True,
                                  tile_position=(64, 0),
                              )
                              aA = apool.tile([P, SBW], BF16, name="aA")
                              nc.scalar.activation(
                                  out=aA, in_=scA, func=AF.Exp, scale=SCALE
                              )
                              aB = apool.tile([P, SBW], BF16, name="aB")
                              nc.vector.tensor_scalar(
                                  out=aB.bitcast(I16),
                                  in0=scB,
                                  scalar1=SCH_S,
                                  scalar2=SCH_B,
                                  op0=OP.mult,
                                  op1=OP.add,
                              )
                              first, last = tci == 0, tci == TC - 1
                              nc.tensor.matmul(
                                  ctx_ps[0:DK, :],
                                  lhsT=v_sb[tci][:, dvA],
                                  rhs=aA,
                                  start=first,
                                  stop=last,
                                  tile_position=(0, 0),
                              )
                              nc.tensor.matmul(
                                  ctx_ps[DK:P, :],
                                  lhsT=v_sb[tci][:, dvB],
                                  rhs=aB,
                                  start=first,
                                  stop=last,
                                  tile_position=(0, 64),
                              )
                              nc.tensor.matmul(
                                  den_ps[0:1, :],
                                  lhsT=ones_t,
                                  rhs=aA,
                                  start=first,
                                  stop=last,
                                  tile_position=(0, 0),
                              )
                              nc.tensor.matmul(
                                  den_ps[32:33, :],
                                  lhsT=ones_t,
                                  rhs=aB,
                                  start=first,
                                  stop=last,
                                  tile_position=(0, 32),
                              )
                          _pending[0] = (p, sb, ctx_ps, den_ps)

                  if _pending[0] is not None:
                      _boundary(*_pending[0])
                      _pending[0] = None

            # ---- phase D: output projection, residual, LN ----
            # Warm-keepers: dependency-free matmuls that bridge the C->D
            # boundary gap so the PE's HAM clock-gate stays at 8/8 for the
            # output projection (a >3.4us idle re-throttles it to 1.2 GHz).
            if "C" in phases and "D" in phases:
                with tc.tile_pool(name="warm", bufs=1, space="PSUM") as wps:
                    wtile = wps.tile([P, 512], F32, name="warm")
                    for _ in range(20):
                        nc.tensor.matmul(
                            wtile, lhsT=ones_r, rhs=bo_sb, start=True, stop=True
                        )

            with (
                tc.tile_pool(name="psum_o", bufs=3, space="PSUM") as pout,
                tc.tile_pool(name="work", bufs=3) as wpool,
                tc.tile_pool(name="qpool", bufs=8) as qpool,
            ):
              if "D" in phases:
                  for st in range(ST):
                      stsl = bass.ts(st, P)
                      po = pout.tile([P, D], F32, name="pout")
                      for c in range(KC):
                          nc.tensor.matmul(
                              po,
                              lhsT=ctxT[c][:, stsl],
                              rhs=W_sb["wo", c],
                              start=(c == 0),
                              stop=False,
                          )
                      nc.tensor.matmul(
                          po, lhsT=ones_r, rhs=bo_sb, start=False, stop=False
                      )
                      # residual: psum += I @ Q via fp32r (full-rate, ~fp32)
                      qres = qpool.tile([P, D], F32R, name="qres")
                      nc.sync.dma_start(out=qres, in_=qf[st * P : (st + 1) * P, :])
                      nc.tensor.matmul(
                          po,
                          lhsT=ident_sb[:],
                          rhs=qres[:],
                          start=False,
                          stop=True,
                      )
                      if st % 2 == 0 and st > 0:
                          # keep PE activity above the HAM re-throttle
                          # threshold across the LN drain gaps
                          wt2 = pout.tile([P, 512], F32, name="pout")
                          nc.tensor.matmul(
                              wt2, lhsT=ones_r, rhs=bo_sb, start=True, stop=True
                          )
                      stats = wpool.tile([P, 6], F32, name="stats")
                      nc.vector.bn_stats(out=stats, in_=po)
                      mv = wpool.tile([P, 2], F32, name="mv")
                      nc.vector.bn_aggr(out=mv, in_=stats)
                      sq = wpool.tile([P, 1], F32, name="sq")
                      nc.scalar.activation(
                          out=sq, in_=mv[:, 1:2], func=AF.Sqrt, bias=eps_sb
                      )
                      rstd = wpool.tile([P, 1], F32, name="rstd")
                      nc.vector.reciprocal(out=rstd, in_=sq)
                      negmu = wpool.tile([P, 1], F32, name="negmu")
                      nc.vector.tensor_scalar(
                          out=negmu,
                          in0=mv[:, 0:1],
                          scalar1=rstd,
                          scalar2=-1.0,
                          op0=OP.mult,
                          op1=OP.mult,
                      )
                      x = wpool.tile([P, D], F32, name="x")
                      nc.scalar.activation(
                          out=x, in_=po, func=AF.Identity, bias=negmu, scale=rstd
                      )
                      nc.vector.tensor_mul(out=x, in0=x, in1=gamma_sb)
                      nc.gpsimd.tensor_tensor(
                          out=x, in0=x, in1=beta_sb, op=OP.add
                      )
                      nc.sync.dma_start(out=out[st * P : (st + 1) * P, :], in_=x)

    _split_excess_waits(nc)
    return nc


_NC_CACHE = None


def _get_program():
    global _NC_CACHE
    if _NC_CACHE is None:
        _NC_CACHE = build_program()
    return _NC_CACHE


def make_in_maps(Q, K, V, Wq, bq, Wk, bk, Wv, bv, Wo, bo, gamma, beta):
    bf = ml_dtypes.bfloat16
    Q = np.asarray(Q, np.float32)
    K = np.asarray(K, np.float32)
    V = np.asarray(V, np.float32)
    wqT = np.ascontiguousarray(np.asarray(Wq, np.float32).T).astype(bf)
    wkT = np.ascontiguousarray(np.asarray(Wk, np.float32).T).astype(bf)
    wvT = np.ascontiguousarray(np.asarray(Wv, np.float32).T).astype(bf)
    woT = np.ascontiguousarray(np.asarray(Wo, np.float32).T).astype(bf)
    bv_r = np.asarray(bv, np.float32).reshape(1, D).astype(bf)
    ident = np.eye(P, dtype=np.float32)
    bo_r = np.asarray(bo, np.float32).reshape(1, D).astype(bf)
    in_maps = []
    for b in range(N_CORES):
        in_maps.append(
            {
                "qf": np.ascontiguousarray(Q[b]),
                "qb": np.ascontiguousarray(Q[b].T).astype(bf),
                "kb": np.ascontiguousarray(K[b].T).astype(bf),
                "vb": np.ascontiguousarray(V[b].T).astype(bf),
                "wq": wqT,
                "wk": wkT,
                "wv": wvT,
                "wo": woT,
                "bq": np.asarray(bq, np.float32),
                "bk": np.asarray(bk, np.float32),
                "bv": bv_r,
                "bo": bo_r,
                "ident": ident,
                "gamma": np.asarray(gamma, np.float32),
                "beta": np.asarray(beta, np.float32),
            }
        )
    return in_maps


def run(in_maps, trace=False, **kw):
    nc = _get_program()
    return run_bass_kernel_spmd(
        nc, in_maps, core_ids=list(range(N_CORES)), trace=trace, **kw
    )


def kernel(**inputs):
    in_maps = make_in_maps(**inputs)
    res = run(in_maps)
    out = np.stack([res.results[b]["out"] for b in range(N_CORES)], axis=0)
    return out.astype(np.float32)

